# revision 30
# baseline (speedup 1.0000x reference)
"""Masked-softmax attention on 8 trn2 NeuronCores.

Reference computation (per batch b):
    att = q @ k                        # [n_q, n_k], k given pre-transposed [d, n_k]
    att = where(mask==0, -1e9, att)
    att = softmax(att, -1) / sqrt(d)
    out = (att @ v).T                  # returned [n_dv, n_q]

Sharding: data-parallel over batch: B=16 -> 2 batches per core x 8 cores.

Host-side, per batch, the key dimension is COMPACTED: masked-out keys
contribute exactly 0 to both the softmax numerator and denominator (the
reference's exp(-1e9 - anything) underflows to +0.0 in fp32), so we gather
only the unmasked columns of k / rows of v, padded up to a multiple of 128
(padding killed by the same -1e9 bias). With a Bernoulli(0.5) mask this
halves the contraction length. Exact, not an approximation.

Device-side plan (per batch). All matmul OPERANDS are bf16 (accumulation
stays fp32 in PSUM): on TRN2's PE both bf16 and f32r run 1 cycle/row at
512-wide moving, but bf16 halves every SBUF fetch and all input DMA, which
removes the SBUF-port contention between the PE's moving-operand stream
and the DVE's e-accumulation traffic (measured: f32r matmul spacing
degrades 232->278 ns when DVE traffic rises; bf16 keeps the PE at rate).
bf16 rounding (~0.2% rms on scores -> ~2.6% per softmax weight averaged
over ~1e3 keys) lands ~1e-3 relative on the output, far inside the 2e-2
gate.

    - Work in the TRANSPOSED score layout S^T[k, q] (k on partitions):
        S^T tile [128k, 512q] = k_slice[d,128k]^T @ qT[d, 512q]  (2 d-chunk accum)
      `k` input [d, n_k] is directly the stationary operand; `q` is transposed
      host-side during sharding so qT[d, n_q] is directly the moving operand.
    - softmax is shift-invariant, so instead of the row max we subtract a
      CONSTANT shift (scores ~ N(0, d) with d=256 -> |s| < ~110 always;
      exp(s-shift) can't overflow and dominant terms can't underflow).
      Mask + shift fold into the scalar-engine exp as a per-partition bias:
        e[k, q] = exp(s + bias_k),  bias_k = -shift - 1e9*(1-mask_k)
    - out^T[dv, q] += v_tile[128k, dv_chunk]^T @ e   (v is directly stationary)
      z[dv, q]    += sixteens[128k, 128]^T @ e       (= 16Z in EVERY partition:
      the all-16s stationary matrix computes the row sum AND broadcasts it,
      folding in the post-softmax 1/sqrt(d)=1/16 scale)
    - out = out^T * (1/z) (DVE approx reciprocal) -> [dv, n_q], the required
      output layout.

DMA schedule: ALL input DMAs for both batches are issued up-front on
large transfers (posting costs the issuing engine ~0.7us each; sub-1KB
row segments halve queue throughput, so chunks stay >=3 tiles wide).
SP HWDGE: q stripes (batch-0 stripe-0 head-of-line) + the padded bias.
ACT HWDGE: batch-0 k in 3 chunk-pairs. gpsimd SWDGE (slow to post, fine
off the critical path): batch-0 v, then all of batch-1. v is pre-
transposed host-side to partition-major [P, nkt*D] so its transfers are
fully contiguous. Outputs (bf16, ~1.7e-3 added rel err) go to a blocked
DRAM layout [NB, NQS, 2, P, QS] so each is one contiguous 1KB-row
transfer on SP/ACT; the host reassembles + casts to f32.

The inner loop is software-pipelined by TWO k-tiles (O(t) emitted after
S(t+2)): the in-order PE queue then never reaches an O matmul before its
~0.9us S->exp chain resolves; one tile of lookahead (~0.86us) was
marginal and produced periodic ~300ns matmul spacings.
"""

import numpy as np
import ml_dtypes

import concourse.bacc as bacc
import concourse.mybir as mybir
import concourse.tile as tile
from concourse.bass_utils import run_bass_kernel_spmd

P = 128          # partitions
D = 256          # d == n_dv
S = 2048         # n_q
NB = 2           # batches per core
QS = 512         # q-stripe width (max matmul moving dim into one PSUM bank)
NQS = S // QS    # 4 q-stripes
NCORES = 8
SHIFT = 60.0     # constant softmax shift (see module docstring)
NWARM = 20       # PE warmup matmuls (fill preamble->first-input window)
QUAD = 16        # e-tiles accumulated on DVE per Z matmul (>=nkt: 1 Z mm/stripe)

F32 = mybir.dt.float32
BF16 = mybir.dt.bfloat16
EXP = mybir.ActivationFunctionType.Exp
MULT = mybir.AluOpType.mult
ADD = mybir.AluOpType.add

BF16NP = ml_dtypes.bfloat16


def build(sk):
    """Build the per-core program. sk = compacted key length (mult of 128)."""
    from contextlib import ExitStack

    nkt = sk // P  # number of k-tiles
    nc = bacc.Bacc()
    qT = nc.declare_dram_parameter("qT", [NB, D, S], BF16, isOutput=False)
    kk = nc.declare_dram_parameter("k", [NB, D, sk], BF16, isOutput=False)
    # v pre-transposed host-side to partition-major [P, nkt*D]: v DMAs
    # become fully contiguous on both sides (4.6KB rows, ~2x throughput)
    vv = nc.declare_dram_parameter("v", [NB, P, (sk // P) * D], BF16, isOutput=False)
    # bias pre-packed host-side as [P, NB*128] so one 1KB-line transfer
    # moves both batches' bias columns
    bb = nc.declare_dram_parameter("bias", [P, NB * P], F32, isOutput=False)
    # blocked output layout: one contiguous [P, QS] block per
    # (batch, stripe, dv-half) so each output DMA moves 2KB rows; the host
    # reassembles [NB, D, S] with a cheap numpy transpose.
    out = nc.declare_dram_parameter("out", [NB, NQS, 2, P, QS], BF16, isOutput=True)

    def chunks(lo, hi, n):
        """Split [lo,hi) into n roughly-equal spans (empty spans dropped)."""
        step = max(1, (hi - lo + n - 1) // n)
        return [(a, min(a + step, hi)) for a in range(lo, hi, step)]

    with tile.TileContext(nc) as tc, ExitStack() as ctx:
        consts = ctx.enter_context(tc.tile_pool(name="consts", bufs=1))
        inp = ctx.enter_context(tc.tile_pool(name="inp", bufs=1))
        epool = ctx.enter_context(tc.tile_pool(name="e", bufs=6))
        opool = ctx.enter_context(tc.tile_pool(name="o", bufs=2))
        zpool = ctx.enter_context(tc.tile_pool(name="z", bufs=2))
        # 5 S banks + 2 O banks + 1 Z = 8. Single-buffered O is safe: the
        # normalize DVE ops of stripe s are emitted before stripe s+1's adds
        # on the in-order DVE, so the banks are free ~2.6us before stripe
        # s+1's first O matmul needs them.
        ps_s = ctx.enter_context(tc.tile_pool(name="ps_s", bufs=5, space="PSUM"))
        ps_o = ctx.enter_context(tc.tile_pool(name="ps_o", bufs=1, space="PSUM"))
        ps_z = ctx.enter_context(tc.tile_pool(name="ps_z", bufs=1, space="PSUM"))

        # (memset can't emit bf16; stage in f32 and DVE-copy to round)
        sixteens_f = consts.tile([P, P], F32)
        nc.vector.memset(sixteens_f, 16.0)
        sixteens = consts.tile([P, P], BF16)
        nc.vector.tensor_copy(sixteens, sixteens_f)

        # Warmup Exp: walrus attaches the implicit ACT table load to the
        # first Exp, which eats its sync-wait slots; give it a dep-free one
        # (also hides the ~2.7us table load under the input DMA fill).
        warm_in = consts.tile([P, 1], F32)
        nc.vector.memset(warm_in, 0.0)
        warm_out = consts.tile([P, 1], F32)
        nc.scalar.activation(warm_out, warm_in, EXP)

        # ---- all input tiles, both batches resident simultaneously
        kts = [
            [inp.tile([P, sk], BF16, tag=f"k{b}{c}", name=f"kt{b}{c}") for c in range(2)]
            for b in range(NB)
        ]
        qts = [
            [inp.tile([P, S], BF16, tag=f"q{b}{c}", name=f"qt{b}{c}") for c in range(2)]
            for b in range(NB)
        ]
        vts = [inp.tile([P, nkt, D], BF16, tag=f"v{b}", name=f"vt{b}") for b in range(NB)]
        bias_all = inp.tile([P, NB * P], F32, tag="bias", name="bias_all")
        biast = [bias_all[:, b * P : b * P + nkt] for b in range(NB)]

        # ---- issue ALL input DMAs up-front, priority-ordered, large
        # full-line transfers only. Posting a descriptor costs the issuing
        # engine ~0.7-0.9us, so critical queues carry few posts.
        def q_stripe_dma(eng, b, s, c):
            eng.dma_start(
                out=qts[b][c][:, s * QS : (s + 1) * QS],
                in_=qT[b, c * P : (c + 1) * P, s * QS : (s + 1) * QS],
            )

        def v_chunk_dma(eng, b, t0, t1):
            eng.dma_start(
                out=vts[b][:, t0:t1, :],
                in_=vv[b, :, t0 * D : t1 * D],
            )

        # SP HWDGE: q stripes, batch 0 stripe 0 first; the (one, padded)
        # bias transfer rides right behind stripe 0, and batch-0's later v
        # chunks slot between q stripes (their O deadlines trail the q ones).
        for c in range(2):
            q_stripe_dma(nc.sync, 0, 0, c)
        nc.sync.dma_start(out=bias_all, in_=bb[:, :])
        for s in range(1, NQS):
            for c in range(2):
                q_stripe_dma(nc.sync, 0, s, c)



        # ACT HWDGE: batch-0 k, both d-halves' leading tiles first so the
        # first S accumulation pair completes ASAP.
        for t0, t1 in chunks(0, nkt, 3):
            for c in range(2):
                nc.scalar.dma_start(
                    out=kts[0][c][:, t0 * P : t1 * P],
                    in_=kk[0, c * P : (c + 1) * P, t0 * P : t1 * P],
                )

        # gpsimd SWDGE: batch-0 v only (O needs v tile 0 by ~11us).
        # Batch-1 inputs are NOT posted here: they are deferred into the
        # compute loop behind data dependencies so their HBM pulls don't
        # compete with batch-0's critical fill window (~8-16us).
        for t0, t1 in chunks(0, nkt, 3):
            v_chunk_dma(nc.gpsimd, 0, t0, t1)
        dummy = consts.tile([P, 1], BF16)

        # PE warmup: dep-free matmuls during the initial DMA fill so the HAM
        # clock gate ramps before the real matmuls start.
        for w in range(NWARM):
            wp = ps_s.tile([P, P], F32, tag="s", name=f"warm{w}")
            nc.tensor.matmul(wp, lhsT=sixteens, rhs=sixteens, start=True, stop=True)

        # ---- compute, one 512-wide q-stripe at a time
        for b in range(NB):
            for s in range(NQS):
                last_stripe = b == NB - 1 and s == NQS - 1
                qoff, qw = s * QS, QS
                qsl = slice(qoff, qoff + qw)
                op0 = ps_o.tile([P, QS], F32, tag="o0", name="op0")[:, :qw]
                op1 = ps_o.tile([P, QS], F32, tag="o1", name="op1")[:, :qw]
                zp = ps_z.tile([P, QS], F32, tag="z", name="zp")[:, :qw]
                acc_e = None
                nacc = 0
                nzmm = (nkt + QUAD - 1) // QUAD
                zi = 0

                def s_exp(t):
                    """Emit the S matmul pair + exp for k-tile t; return e."""
                    ksl = slice(t * P, (t + 1) * P)
                    sp = ps_s.tile([P, QS], F32, tag="s", name="sp")[:, :qw]
                    nc.tensor.matmul(
                        sp, lhsT=kts[b][0][:, ksl], rhs=qts[b][0][:, qsl],
                        start=True, stop=False,
                    )
                    nc.tensor.matmul(
                        sp, lhsT=kts[b][1][:, ksl], rhs=qts[b][1][:, qsl],
                        start=False, stop=True,
                    )
                    e = epool.tile([P, QS], BF16, tag="e", name="e")[:, :qw]
                    nc.scalar.activation(e, sp, EXP, bias=biast[b][:, t : t + 1])
                    return e

                # Software-pipelined by FOUR tiles: O(t) is emitted after
                # S(t+4). Two tiles (~1.7us) covers the ~0.9us S->exp
                # latency; the extra depth lets the PE scoreboard keep
                # running S matmuls ahead while early v chunks are still in
                # flight. ps_s bufs=5 holds sp(t)..sp(t+4).
                DEPTH = 4
                pipe = [s_exp(tt) for tt in range(min(DEPTH, nkt))]
                for t in range(nkt):
                    e = pipe.pop(0)
                    if t + DEPTH < nkt:
                        pipe.append(s_exp(t + DEPTH))
                    first, last = t == 0, t == nkt - 1
                    nc.tensor.matmul(
                        op0, lhsT=vts[b][:, t, 0:P], rhs=e, start=first, stop=last,
                    )
                    nc.tensor.matmul(
                        op1, lhsT=vts[b][:, t, P : 2 * P], rhs=e, start=first, stop=last,
                    )
                    # Z: a running DVE accumulator sums QUAD e-tiles so only
                    # ceil(nkt/QUAD) Z matmuls run (PE cycles -> idle DVE).
                    # On the final stripe the last e-tile goes into its own
                    # accumulating Z matmul so Z completes right after the
                    # final exp instead of behind one more DVE add.
                    if last_stripe and last and acc_e is not None:
                        nc.tensor.matmul(
                            zp, lhsT=sixteens, rhs=acc_e, start=zi == 0, stop=False,
                        )
                        nc.tensor.matmul(
                            zp, lhsT=sixteens, rhs=e, start=False, stop=True,
                        )
                        zi = nzmm
                        acc_e, nacc = None, 0
                        continue
                    if acc_e is None:
                        acc_e, nacc = e, 1
                    else:
                        na = epool.tile([P, QS], BF16, tag="ep", name="na")[:, :qw]
                        nc.vector.tensor_tensor(na, acc_e, e, ADD)
                        acc_e = na
                        nacc += 1
                    if nacc == QUAD or t == nkt - 1:
                        nc.tensor.matmul(
                            zp, lhsT=sixteens, rhs=acc_e,
                            start=zi == 0, stop=zi == nzmm - 1,
                        )
                        zi += 1
                        acc_e, nacc = None, 0
                # normalize: out = out_unnorm * (1/(16Z)); zp already holds
                # 16Z in every partition. ~18-bit reciprocal, 5x faster than
                # exact; z is far from denorm/inf so approx edge cases can't
                # hit. Processed in chunks so the tail (recip -> mult -> DMA)
                # pipelines; the final stripe uses finer chunks to shorten
                # the drain.
                zbs = zpool.tile([P, QS], F32, tag="zbs", name="zbs")[:, :qw]
                o0 = opool.tile([P, QS], BF16, tag="so0", name="o0")[:, :qw]
                o1 = opool.tile([P, QS], BF16, tag="so1", name="o1")[:, :qw]
                if b == 0 and s == 0:
                    # deferred batch-1 q: the SP engine reaches these posts
                    # only after stripe (0,0)'s out-DMA wait resolves
                    for s2 in range(NQS):
                        for c in range(2):
                            q_stripe_dma(nc.sync, 1, s2, c)
                if b == 0 and s == 1:
                    # deferred batch-1 k/v: the dummy copy makes gpsimd wait
                    # for stripe (0,1)'s o0 before posting
                    nc.gpsimd.tensor_copy(dummy, o0[:, 0:1])
                    for c in range(2):
                        nc.gpsimd.dma_start(
                            out=kts[1][c], in_=kk[1, c * P : (c + 1) * P, :]
                        )
                    for t0, t1 in chunks(0, nkt, 2):
                        v_chunk_dma(nc.gpsimd, 1, t0, t1)
                if last_stripe:
                    # Drain chain: DVE handles recip + the o0 stream; the o1
                    # stream goes ACT Copy (PSUM->SBUF; Copy shares the Exp
                    # table so no table reload) + Pool multiply, in parallel
                    # with DVE. Halved so mult/DMA pipeline.
                    o1s = opool.tile([P, QS], F32, tag="so1c", name="o1s")[:, :qw]
                    COPY = mybir.ActivationFunctionType.Copy
                    nc.scalar.activation(o1s, op1, COPY)
                    hw2 = qw // 2
                    for h in range(2):
                        hs = slice(h * hw2, (h + 1) * hw2)
                        nc.vector.reciprocal_approx_fast(out=zbs[:, hs], in_=zp[:, hs])
                        nc.vector.tensor_tensor(o0[:, hs], op0[:, hs], zbs[:, hs], MULT)
                        nc.sync.dma_start(out=out[b, s, 0][:, hs], in_=o0[:, hs])
                        nc.gpsimd.tensor_tensor(o1[:, hs], o1s[:, hs], zbs[:, hs], MULT)
                        nc.scalar.dma_start(out=out[b, s, 1][:, hs], in_=o1[:, hs])
                else:
                    nc.vector.reciprocal_approx_fast(out=zbs, in_=zp)
                    nc.vector.tensor_tensor(o0, op0, zbs, MULT)
                    nc.sync.dma_start(out=out[b, s, 0], in_=o0)
                    nc.vector.tensor_tensor(o1, op1, zbs, MULT)
                    nc.scalar.dma_start(out=out[b, s, 1], in_=o1)

    return nc


def make_in_maps(q, k, v, mask):
    """Shard over batch; transpose q; compact the key dim to unmasked keys."""
    q = np.asarray(q, dtype=np.float32)
    k = np.asarray(k, dtype=np.float32)
    v = np.asarray(v, dtype=np.float32)
    mask = np.asarray(mask, dtype=np.int32).reshape(len(q), -1)

    B = len(q)
    idxs = [np.nonzero(mask[b])[0] for b in range(B)]
    n_eff = max((len(ix) for ix in idxs), default=1)
    sk = max(P, ((n_eff + P - 1) // P) * P)  # padded compacted key length

    kg = np.zeros((B, D, sk), dtype=np.float32)
    vg = np.zeros((B, sk, D), dtype=np.float32)
    # exp bias: -SHIFT for real keys, -1e9 for padding (kills it exactly),
    # laid out [P, sk//P] partition-major to match the k-tile slicing
    bg = np.full((B, sk), -1.0e9, dtype=np.float32)
    for b in range(B):
        ix = idxs[b]
        kg[b, :, : len(ix)] = k[b][:, ix]
        vg[b, : len(ix)] = v[b][ix]
        bg[b, : len(ix)] = -SHIFT
    bgt = bg.reshape(B, sk // P, P).transpose(0, 2, 1)  # [B, P, nkt]
    # v partition-major: vgt[b, p, t*D+d] = vg[b, t*128+p, d]
    vgt = vg.reshape(B, sk // P, P, D).transpose(0, 2, 1, 3).reshape(B, P, -1)
    bgp = np.zeros((B, P, P), dtype=np.float32)  # rows padded to 512B lines
    bgp[:, :, : sk // P] = bgt
    # pack per-core as [P, NB*128]: core i gets batches i*NB..i*NB+NB-1
    bgq = bgp.transpose(1, 0, 2).reshape(P, B * P)

    in_maps = []
    for i in range(NCORES):
        sl = slice(i * NB, (i + 1) * NB)
        in_maps.append(
            {
                "qT": np.ascontiguousarray(
                    np.transpose(q[sl], (0, 2, 1)).astype(BF16NP)
                ),
                "k": np.ascontiguousarray(kg[sl].astype(BF16NP)),
                "v": np.ascontiguousarray(vgt[sl].astype(BF16NP)),
                "bias": np.ascontiguousarray(
                    bgq[:, i * NB * P : (i + 1) * NB * P]
                ),
            }
        )
    return in_maps, sk


def run(q, k, v, mask, **kwargs):
    in_maps, sk = make_in_maps(q, k, v, mask)
    nc = build(sk)
    nc.finalize()  # run the Bacc pass pipeline (reg alloc, wait splitting)
    res = run_bass_kernel_spmd(nc, in_maps, list(range(NCORES)), **kwargs)
    # device layout [NB, NQS, 2, P, QS] -> [NB, D, S]
    out = np.concatenate(
        [
            r["out"].transpose(0, 2, 3, 1, 4).reshape(NB, D, S)
            for r in res.results
        ],
        axis=0,
    ).astype(np.float32)
    return out, res


def kernel(q, k, v, mask):
    out, _ = run(q, k, v, mask)
    return out
